# revision 34
# baseline (speedup 1.0000x reference)
"""Trainium2 Bass kernel for Bahdanau-style attention (nn_Attention).

Reference computation (per batch b):
    dec_proj[a]   = decoder_hidden[b] @ w_dec + attn_b            # [A]
    enc_proj[s,a] = encoder_outputs[b] @ w_enc                    # [S, A]
    energy        = tanh(dec_proj + enc_proj)                     # [S, A]
    att[s]        = energy @ v_w                                  # [S]
    attw          = softmax(att)                                  # [S]  (mask is all-ones)
    context[e]    = attw @ encoder_outputs[b]                     # [2H]

Shapes: B=64, S=512, H=1024, A=1024, E=2H=2048.

Strategy: pure data parallelism over batch — 8 batches per NeuronCore, no
collectives.  Host pre-transposes encoder_outputs to [B, E, S] so the big
matmul runs as enc_projT[a,s] = w_encT @ encT with the contraction dim (e)
on partitions for both operands — zero on-device transposes.  The dominant
matmul runs in fp8 (e4m3) with DoubleRow perf mode (two k-tiles per
instruction); inputs are pre-scaled (enc x16, w_enc x64) to stay out of the
fp8 subnormal range and the 1/1024 descale folds into the tanh activation's
scale parameter.  The tanh+bias fuses into one ScalarE activation (dec_proj
is a per-partition bias in this layout), the v-dot is a small TensorE matmul
with the softmax entirely off the TensorE critical path, and the context
reduction runs over a separate bf16 copy of the encoder tile, split between
fused multiply+accumulate (scalar_tensor_tensor) on VectorE and multiply (2x
bf16 on VectorE) + free-dim reduce (Identity activation with accum_out) on
ScalarE, with the per-batch tail software-pipelined one batch behind the
TensorE stream.
"""

import sys

for _p in ("/opt/trn_rl_repo",):
    if _p not in sys.path:
        sys.path.insert(0, _p)

import numpy as np
import ml_dtypes


def _seed_ntff_hook():
    """This image's antenv lacks axon_hooks; seed it so any trace=True /
    BASS_TRACE path degrades gracefully (and profiles when possible)."""
    if "antenv.axon_hooks" in sys.modules:
        return
    import types
    hook = None
    try:
        sys.path.insert(0, "/root/.axon_site")
        from trn_agent_boot.trn_boot import _ntff_profile_via_ctypes
        hook = _ntff_profile_via_ctypes("/opt/axon/libaxon_pjrt.so")
    except Exception:
        hook = None
    mod = types.ModuleType("antenv.axon_hooks")
    mod.get_axon_ntff_profile_hook = lambda: hook
    mod.set_axon_ntff_profile_hook = lambda h: None
    sys.modules["antenv.axon_hooks"] = mod


_seed_ntff_hook()

import concourse.bass as bass
import concourse.mybir as mybir
import concourse.tile as tile
from concourse import bacc
from concourse.bass import ts
from concourse.bass_utils import run_bass_kernel_spmd

BF16 = mybir.dt.bfloat16
F32 = mybir.dt.float32
FP8 = mybir.dt.float8e4
NP_BF16 = ml_dtypes.bfloat16
NP_FP8 = mybir.dt.np(FP8)

N_CORES = 8
B, S, H, A = 64, 512, 1024, 1024
E = 2 * H                 # 2048
BSH = B // N_CORES        # 8 batches per core
AT = A // 128             # 8 a-tiles
KE = E // 128             # 16 e contraction tiles
KH = H // 128             # 8 h contraction tiles

USE_FP8 = True
ATT_FP8 = False
V_SCALE = 256.0
ENC_SCALE = 16.0
W_SCALE = 64.0
DESCALE = 1.0 / (ENC_SCALE * W_SCALE)


def build_nc():
    nc = bacc.Bacc("TRN2", target_bir_lowering=False, debug=False,
                   num_devices=N_CORES)

    enc8 = nc.dram_tensor("enc8", [BSH, E, S], FP8 if USE_FP8 else BF16,
                          kind="ExternalInput").ap()
    encb = nc.dram_tensor("encb", [BSH, E, S], BF16, kind="ExternalInput").ap()
    w_dec = nc.dram_tensor("w_dec", [H, A], FP8 if USE_FP8 else BF16,
                           kind="ExternalInput").ap()
    w_enc = nc.dram_tensor("w_enc", [E, A], FP8 if USE_FP8 else BF16,
                           kind="ExternalInput").ap()
    # dh (64B fp8) + attn_b (32B f32) + v_w (16B bf16) byte-packed: 1 DMA
    consts = nc.dram_tensor("consts", [128, 112], mybir.dt.uint8,
                            kind="ExternalInput").ap()
    encn = nc.dram_tensor("encn", [S, E], BF16, kind="ExternalInput").ap()
    out_ctx = nc.dram_tensor("out_ctx", [128, BSH * KE], F32,
                             kind="ExternalOutput").ap()
    out_ctx7 = nc.dram_tensor("out_ctx7", [1, E], F32,
                              kind="ExternalOutput").ap()
    out_attw = nc.dram_tensor("out_attw", [BSH, S], F32,
                              kind="ExternalOutput").ap()

    Tanh = mybir.ActivationFunctionType.Tanh
    Exp = mybir.ActivationFunctionType.Exp
    Ident = mybir.ActivationFunctionType.Identity
    X = mybir.AxisListType.X
    mult = mybir.AluOpType.mult
    amax = mybir.AluOpType.max
    DR = mybir.MatmulPerfMode.DoubleRow if USE_FP8 else None
    KP = KE // 2 if USE_FP8 else KE     # k-steps of the big matmul

    with tile.TileContext(nc) as tc:
        with (
            tc.tile_pool(name="singles", bufs=1) as singles,
            tc.tile_pool(name="enc8p", bufs=3) as enc8p,
            tc.tile_pool(name="encbp", bufs=2) as encbp,
            tc.tile_pool(name="energyp", bufs=3) as energyp,
            tc.tile_pool(name="wbcp", bufs=2) as wbcp,
            tc.tile_pool(name="sttp", bufs=4) as sttp,
            tc.tile_pool(name="smallp", bufs=4) as smallp,
            tc.tile_pool(name="ps_e", bufs=3, space="PSUM") as ps_e,
            tc.tile_pool(name="ps_att", bufs=2, space="PSUM") as ps_att,
            tc.tile_pool(name="ps_misc", bufs=2, space="PSUM") as ps_misc,
        ):
            # --- resident constants, ordered so first consumers land first
            consts_sb = singles.tile([128, 112], mybir.dt.uint8)
            nc.sync.dma_start(out=consts_sb[:], in_=consts[:, :])
            dh_sb = consts_sb[:, 0:64].bitcast(
                FP8 if USE_FP8 else BF16).rearrange("p (t b) -> p t b", b=BSH)
            attnb_sb = consts_sb[:, 64:96].bitcast(F32)
            v_sb = consts_sb[:, 96:112].bitcast(BF16)
            ones_sb = singles.tile([1, 128], BF16)
            nc.vector.memset(ones_sb[:], 1.0)

            wd_sb = singles.tile([128, KH, A], FP8 if USE_FP8 else BF16)
            wd_dram = w_dec.rearrange("(t p) a -> p t a", p=128)
            nc.sync.dma_start(out=wd_sb[:], in_=wd_dram[:, :, :])

            we_sb = singles.tile([128, KE, A], FP8 if USE_FP8 else BF16)
            we_dram = w_enc.rearrange("(t p) a -> p t a", p=128)

            ctx_sb = singles.tile([128, BSH * KE], F32)
            dpT_sb = singles.tile([128, AT, BSH], F32)

            # --- dec_projT[a, b] = w_dec.T @ dh  (+ attn_b) --------------
            KPH = KH // 2 if USE_FP8 else KH
            for t in range(AT):
                ps = ps_misc.tile([128, BSH], F32, bufs=2)
                if USE_FP8:
                    for k in range(KPH):
                        nc.tensor.matmul(ps[:],
                                         wd_sb[:, 2 * k:2 * k + 2, ts(t, 128)],
                                         dh_sb[:, 2 * k:2 * k + 2, :],
                                         start=(k == 0), stop=(k == KPH - 1),
                                         perf_mode=DR)
                else:
                    for k in range(KH):
                        nc.tensor.matmul(ps[:], wd_sb[:, k, ts(t, 128)],
                                         dh_sb[:, k, :],
                                         start=(k == 0), stop=(k == KH - 1))
                nc.scalar.activation(out=dpT_sb[:, t, :], in_=ps[:],
                                     func=Ident, bias=attnb_sb[:, t:t + 1],
                                     scale=DESCALE if USE_FP8 else 1.0)

            # --- per-batch pipeline -------------------------------------
            # The post-softmax work of batch b (weights broadcast + context
            # reduction) is emitted in the middle of batch b+1's matmul
            # stream, so the TensorE ones-broadcast never stalls on the
            # softmax and the Vector/Scalar engines drain steadily.
            def make_context_stage(b, eb_sb, rzvec, p_bf):
                def emit():
                    # broadcast normalized attw across partitions in one PE
                    # matmul: wbc[p, s] = rzvec[p] * p_bf[s] = attw[s]
                    wbc = wbcp.tile([128, S], BF16)
                    ps_w = ps_misc.tile([128, S], F32, bufs=1)
                    nc.tensor.matmul(ps_w[:], rzvec[:1, :], p_bf[:1, :],
                                     start=True, stop=True)
                    nc.vector.tensor_copy(out=wbc[:], in_=ps_w[:])
                    # context[e] = sum_s attw[s] * enc[e, s]: most tiles use
                    # the fused multiply+reduce (scalar_tensor_tensor, 1x
                    # rate) on VectorE; the rest use a 2x-rate bf16 multiply
                    # on VectorE plus the free-dim reduction (Identity +
                    # accum_out) on ScalarE.  The last batch is the serial
                    # tail, so split it evenly.
                    n_stt = {BSH - 2: KE}.get(b, 12)
                    # emit the ScalarE-reduced tiles first so that lane
                    # starts as early as possible (matters for the tail)
                    for k in list(range(n_stt, KE)) + list(range(n_stt)):
                        acc = ctx_sb[:, b * KE + k: b * KE + k + 1]
                        scratch = sttp.tile([128, S], BF16)
                        if k < n_stt:
                            nc.vector.scalar_tensor_tensor(
                                out=scratch[:], in0=eb_sb[:, k, :], scalar=1.0,
                                in1=wbc[:], op0=mult, op1=mult, accum_out=acc)
                        else:
                            nc.vector.tensor_mul(scratch[:], eb_sb[:, k, :],
                                                 wbc[:])
                            nc.scalar.activation(out=scratch[:],
                                                 in_=scratch[:], func=Ident,
                                                 bias=0.0, scale=1.0,
                                                 accum_out=acc)
                    nc.sync.dma_start(out=out_ctx[:, ts(b, KE)],
                                      in_=ctx_sb[:, ts(b, KE)])
                return emit

            encn_sb = singles.tile([128, S // 128, E], BF16)

            def make_tail_context_stage(rz, p_bf):
                # Last batch: the TensorE is idle once its att matmuls are
                # done, and it is much faster than the Vector/Scalar lanes
                # at this reduction.  Transpose the exp row into columns
                # with four K=1 matmuls (rhs = scalar one), then contract
                # over s against a natural-layout [s, e] copy of this
                # batch's encoder; 1/Z folds into the psum-drain copies.
                def emit():
                    tp_ps = ps_misc.tile([128, 4], F32, bufs=1, tag="ps_w")
                    for st in range(4):
                        nc.tensor.matmul(tp_ps[:, st:st + 1],
                                         p_bf[0:1, ts(st, 128)],
                                         ones_sb[0:1, 0:1],
                                         start=(st == 0), stop=(st == 3),
                                         skip_group_check=True)
                    pn_col = smallp.tile([128, 4], BF16)
                    nc.vector.tensor_copy(out=pn_col[:], in_=tp_ps[:])
                    ctx7_sb = smallp.tile([1, E], F32)
                    Copy = mybir.ActivationFunctionType.Copy
                    for ce in range(4):
                        ps = ps_e.tile([128, S], F32)
                        for st in range(4):
                            nc.tensor.matmul(ps[0:1, :], pn_col[:, st:st + 1],
                                             encn_sb[:, st, ts(ce, S)],
                                             start=(st == 0), stop=(st == 3),
                                             skip_group_check=True)
                        nc.scalar.activation(out=ctx7_sb[0:1, ts(ce, S)],
                                             in_=ps[0:1, :], func=Copy,
                                             scale=rz[:])
                    nc.sync.dma_start(out=out_ctx7[:, :], in_=ctx7_sb[:])
                return emit

            pending_context = None
            for b in range(BSH):
                e8_sb = enc8p.tile([128, KE, S], FP8 if USE_FP8 else BF16)
                e8_dram = enc8[b].rearrange("(t p) s -> p t s", p=128)
                if b == 0:
                    # first batch: interleave the w_enc pair-tiles with the
                    # encoder quarters on the queue so the first a-tile's
                    # k-loop can track arrivals instead of waiting for the
                    # full weight block
                    for q in range(4):
                        nc.sync.dma_start(out=e8_sb[:, ts(q, KE // 4), :],
                                          in_=e8_dram[:, ts(q, KE // 4), :])
                        nc.sync.dma_start(out=we_sb[:, ts(q, 4), :],
                                          in_=we_dram[:, ts(q, 4), :])
                else:
                    for q in range(2):
                        nc.sync.dma_start(out=e8_sb[:, ts(q, KE // 2), :],
                                          in_=e8_dram[:, ts(q, KE // 2), :])
                if b < BSH - 1:
                    eb_sb = encbp.tile([128, KE, S], BF16)
                    eb_dram = encb[b].rearrange("(t p) s -> p t s", p=128)
                    for q in range(2):
                        nc.sync.dma_start(out=eb_sb[:, ts(q, KE // 2), :],
                                          in_=eb_dram[:, ts(q, KE // 2), :])
                else:
                    eb_sb = None
                if 3 <= b <= 6:
                    q = b - 3
                    encn_r = encn.rearrange("(st p) e -> p st e", p=128)
                    nc.sync.dma_start(out=encn_sb[:, q, :],
                                      in_=encn_r[:, q, :])

                energy_sb = energyp.tile([128, AT, S],
                                         FP8 if ATT_FP8 else BF16)
                ps_a = ps_att.tile([1, S], F32)
                for t in range(AT):
                    ps = ps_e.tile([128, S], F32)
                    if USE_FP8:
                        for k in range(KP):
                            nc.tensor.matmul(
                                ps[:], we_sb[:, 2 * k:2 * k + 2, ts(t, 128)],
                                e8_sb[:, 2 * k:2 * k + 2, :],
                                start=(k == 0), stop=(k == KP - 1),
                                perf_mode=DR)
                    else:
                        for k in range(KE):
                            nc.tensor.matmul(ps[:], we_sb[:, k, ts(t, 128)],
                                             e8_sb[:, k, :],
                                             start=(k == 0), stop=(k == KE - 1))
                    # energy = tanh(enc_proj * descale + dec_proj)
                    nc.scalar.activation(out=energy_sb[:, t, :], in_=ps[:],
                                         func=Tanh,
                                         bias=dpT_sb[:, t, b:b + 1],
                                         scale=DESCALE if USE_FP8 else 1.0)
                    # att[s] += v[a-tile] . energy[a-tile, s]
                    if ATT_FP8:
                        if t % 2 == 1:
                            tp = t // 2
                            nc.tensor.matmul(
                                ps_a[:], v_sb[:, 2 * tp:2 * tp + 2, 0:1],
                                energy_sb[:, 2 * tp:2 * tp + 2, :],
                                start=(tp == 0), stop=(tp == AT // 2 - 1),
                                perf_mode=DR, skip_group_check=True)
                    else:
                        nc.tensor.matmul(ps_a[:], v_sb[:, t:t + 1],
                                         energy_sb[:, t, :],
                                         start=(t == 0), stop=(t == AT - 1),
                                         skip_group_check=True)
                    if t == 1 and pending_context is not None:
                        pending_context()
                        pending_context = None

                # --- softmax over S on [1, 512] -------------------------
                # No max-subtraction: |att| <= sum|v_w| ~ 18, exp cannot
                # overflow in fp32, so exp reads the PSUM scores directly
                # and emits bf16 (its fp32 row-sum comes via accum_out).
                p_bf = smallp.tile([1, S], BF16)
                zs = smallp.tile([1, 1], F32)
                nc.scalar.activation(out=p_bf[:], in_=ps_a[:], func=Exp,
                                     bias=0.0,
                                     scale=(1.0 / V_SCALE) if ATT_FP8 else 1.0,
                                     accum_out=zs[:])
                rz = smallp.tile([1, 1], F32)
                nc.vector.reciprocal(out=rz[:], in_=zs[:])
                # 1/Z folded into the broadcast matmul's stationary vector
                rzvec = smallp.tile([1, 128], BF16)
                nc.vector.tensor_scalar_mul(out=rzvec[:], in0=ones_sb[:],
                                            scalar1=rz[:])
                # fp32 normalized weights only feed the DMA'd output
                pn = smallp.tile([1, S], F32)
                nc.vector.tensor_scalar_mul(out=pn[:], in0=p_bf[:], scalar1=rz[:])
                nc.sync.dma_start(out=out_attw[b:b + 1, :], in_=pn[:])

                if b < BSH - 1:
                    pending_context = make_context_stage(b, eb_sb, rzvec, p_bf)
                else:
                    pending_context = make_tail_context_stage(rz, p_bf)

            pending_context()

    nc.compile()
    return nc


_NC_CACHE = None


def _get_nc():
    global _NC_CACHE
    if _NC_CACHE is None:
        _NC_CACHE = build_nc()
    return _NC_CACHE


def _pack_dh(dh_i):
    # [BSH, H] -> [128, KH*BSH] with [p, t*BSH+b] = dh_i[b, t*128+p]
    arr = dh_i.T.reshape(KH, 128, BSH).transpose(1, 0, 2).reshape(128, KH * BSH)
    if USE_FP8:
        return np.clip(arr * ENC_SCALE, -224, 224).astype(NP_FP8, order="C")
    return arr.astype(NP_BF16, order="C")


def _stage_inputs(decoder_hidden, encoder_outputs, attn_w, attn_b, v_w):
    """Shard + lay out host-side. Returns in_maps for the 8 cores."""
    dh = np.asarray(decoder_hidden, np.float32)
    enc = np.asarray(encoder_outputs, np.float32)
    w = np.asarray(attn_w, np.float32)
    if USE_FP8:
        wd = np.clip(w[:H] * W_SCALE, -224, 224).astype(NP_FP8)    # [H, A]
    else:
        wd = w[:H].astype(NP_BF16)
    if USE_FP8:
        we = np.clip(w[H:] * W_SCALE, -224, 224).astype(NP_FP8)
    else:
        we = w[H:].astype(NP_BF16)
    ab = np.ascontiguousarray(
        np.asarray(attn_b, np.float32).reshape(AT, 128).T)      # [128, AT]
    vv = np.ascontiguousarray(
        np.asarray(v_w, np.float32).reshape(AT, 128).T).astype(NP_BF16)

    in_maps = []
    for i in range(N_CORES):
        sl = slice(i * BSH, (i + 1) * BSH)
        enc_t = enc[sl].transpose(0, 2, 1)
        if USE_FP8:
            enc8_i = np.clip(enc_t * ENC_SCALE, -224, 224).astype(
                NP_FP8, order="C")
        else:
            enc8_i = enc_t.astype(NP_BF16, order="C")
        consts_i = np.zeros((128, 112), np.uint8)
        consts_i[:, 0:64] = _pack_dh(dh[sl]).view(np.uint8)
        consts_i[:, 64:96] = ab.view(np.uint8)
        consts_i[:, 96:112] = vv.view(np.uint8)
        in_maps.append({
            "encn": enc[sl][BSH - 1].astype(NP_BF16, order="C"),
            "enc8": enc8_i,
            "encb": enc_t.astype(NP_BF16, order="C"),
            "w_dec": wd,
            "w_enc": we,
            "consts": consts_i,
        })
    return in_maps


def run(inputs, trace=False, **run_kwargs):
    """Compile (cached), execute on 8 cores, reassemble full outputs.

    Returns ((context, attention_weights), BassKernelResults).
    """
    nc = _get_nc()
    in_maps = _stage_inputs(
        inputs["decoder_hidden"], inputs["encoder_outputs"],
        inputs["attn_w"], inputs["attn_b"], inputs["v_w"])
    res = run_bass_kernel_spmd(nc, in_maps, core_ids=list(range(N_CORES)),
                               trace=trace, **run_kwargs)
    context = np.empty((B, E), np.float32)
    attw = np.empty((B, S), np.float32)
    for i, r in enumerate(res.results):
        sl = slice(i * BSH, (i + 1) * BSH)
        # out_ctx[p, b*KE + k] == context[b, k*128 + p]
        context[sl] = (r["out_ctx"].reshape(128, BSH, KE)
                       .transpose(1, 2, 0).reshape(BSH, E))
        context[i * BSH + BSH - 1] = r["out_ctx7"][0]
        attw[sl] = r["out_attw"]
    return (context, attw), res


def kernel(**inputs):
    (context, attw), _ = run(inputs, trace=False)
    return context, attw


# revision 37
# speedup vs baseline: 1.0175x; 1.0175x over previous
"""Trainium2 Bass kernel for Bahdanau-style attention (nn_Attention).

Reference computation (per batch b):
    dec_proj[a]   = decoder_hidden[b] @ w_dec + attn_b            # [A]
    enc_proj[s,a] = encoder_outputs[b] @ w_enc                    # [S, A]
    energy        = tanh(dec_proj + enc_proj)                     # [S, A]
    att[s]        = energy @ v_w                                  # [S]
    attw          = softmax(att)                                  # [S]  (mask is all-ones)
    context[e]    = attw @ encoder_outputs[b]                     # [2H]

Shapes: B=64, S=512, H=1024, A=1024, E=2H=2048.

Strategy: pure data parallelism over batch — 8 batches per NeuronCore, no
collectives.  Host pre-transposes encoder_outputs to [B, E, S] so the big
matmul runs as enc_projT[a,s] = w_encT @ encT with the contraction dim (e)
on partitions for both operands — zero on-device transposes.  The dominant
matmul runs in fp8 (e4m3) with DoubleRow perf mode (two k-tiles per
instruction); inputs are pre-scaled (enc x16, w_enc x64) to stay out of the
fp8 subnormal range and the 1/1024 descale folds into the tanh activation's
scale parameter.  The tanh+bias fuses into one ScalarE activation (dec_proj
is a per-partition bias in this layout), the v-dot is a small TensorE matmul
with the softmax entirely off the TensorE critical path, and the context
reduction runs over a separate bf16 copy of the encoder tile, split between
fused multiply+accumulate (scalar_tensor_tensor) on VectorE and multiply (2x
bf16 on VectorE) + free-dim reduce (Identity activation with accum_out) on
ScalarE, with the per-batch tail software-pipelined one batch behind the
TensorE stream.
"""

import sys

for _p in ("/opt/trn_rl_repo",):
    if _p not in sys.path:
        sys.path.insert(0, _p)

import numpy as np
import ml_dtypes


def _seed_ntff_hook():
    """This image's antenv lacks axon_hooks; seed it so any trace=True /
    BASS_TRACE path degrades gracefully (and profiles when possible)."""
    if "antenv.axon_hooks" in sys.modules:
        return
    import types
    hook = None
    try:
        sys.path.insert(0, "/root/.axon_site")
        from trn_agent_boot.trn_boot import _ntff_profile_via_ctypes
        hook = _ntff_profile_via_ctypes("/opt/axon/libaxon_pjrt.so")
    except Exception:
        hook = None
    mod = types.ModuleType("antenv.axon_hooks")
    mod.get_axon_ntff_profile_hook = lambda: hook
    mod.set_axon_ntff_profile_hook = lambda h: None
    sys.modules["antenv.axon_hooks"] = mod


_seed_ntff_hook()

import concourse.bass as bass
import concourse.mybir as mybir
import concourse.tile as tile
from concourse import bacc
from concourse.bass import ts
from concourse.bass_utils import run_bass_kernel_spmd

BF16 = mybir.dt.bfloat16
F32 = mybir.dt.float32
FP8 = mybir.dt.float8e4
NP_BF16 = ml_dtypes.bfloat16
NP_FP8 = mybir.dt.np(FP8)

N_CORES = 8
B, S, H, A = 64, 512, 1024, 1024
E = 2 * H                 # 2048
BSH = B // N_CORES        # 8 batches per core
AT = A // 128             # 8 a-tiles
KE = E // 128             # 16 e contraction tiles
KH = H // 128             # 8 h contraction tiles

USE_FP8 = True
ATT_FP8 = False
V_SCALE = 256.0
ENC_SCALE = 16.0
W_SCALE = 64.0
DESCALE = 1.0 / (ENC_SCALE * W_SCALE)


def build_nc():
    nc = bacc.Bacc("TRN2", target_bir_lowering=False, debug=False,
                   num_devices=N_CORES)

    enc8 = nc.dram_tensor("enc8", [BSH, E, S], FP8 if USE_FP8 else BF16,
                          kind="ExternalInput").ap()
    encb = nc.dram_tensor("encb", [BSH, E, S], BF16, kind="ExternalInput").ap()
    w_dec = nc.dram_tensor("w_dec", [H, A], FP8 if USE_FP8 else BF16,
                           kind="ExternalInput").ap()
    w_enc = nc.dram_tensor("w_enc", [E, A], FP8 if USE_FP8 else BF16,
                           kind="ExternalInput").ap()
    # dh (64B fp8) + attn_b (32B f32) + v_w (16B bf16) byte-packed: 1 DMA
    consts = nc.dram_tensor("consts", [128, 112], mybir.dt.uint8,
                            kind="ExternalInput").ap()
    encn = nc.dram_tensor("encn", [S, E], BF16, kind="ExternalInput").ap()
    out_ctx = nc.dram_tensor("out_ctx", [128, BSH * KE], F32,
                             kind="ExternalOutput").ap()
    out_ctx7 = nc.dram_tensor("out_ctx7", [1, E], F32,
                              kind="ExternalOutput").ap()
    out_attw = nc.dram_tensor("out_attw", [BSH, S], F32,
                              kind="ExternalOutput").ap()

    Tanh = mybir.ActivationFunctionType.Tanh
    Exp = mybir.ActivationFunctionType.Exp
    Ident = mybir.ActivationFunctionType.Identity
    X = mybir.AxisListType.X
    mult = mybir.AluOpType.mult
    amax = mybir.AluOpType.max
    DR = mybir.MatmulPerfMode.DoubleRow if USE_FP8 else None
    KP = KE // 2 if USE_FP8 else KE     # k-steps of the big matmul

    with tile.TileContext(nc) as tc:
        with (
            tc.tile_pool(name="singles", bufs=1) as singles,
            tc.tile_pool(name="enc8p", bufs=3) as enc8p,
            tc.tile_pool(name="encbp", bufs=2) as encbp,
            tc.tile_pool(name="energyp", bufs=3) as energyp,
            tc.tile_pool(name="wbcp", bufs=2) as wbcp,
            tc.tile_pool(name="sttp", bufs=4) as sttp,
            tc.tile_pool(name="smallp", bufs=4) as smallp,
            tc.tile_pool(name="ps_e", bufs=3, space="PSUM") as ps_e,
            tc.tile_pool(name="ps_att", bufs=2, space="PSUM") as ps_att,
            tc.tile_pool(name="ps_misc", bufs=2, space="PSUM") as ps_misc,
        ):
            # --- resident constants, ordered so first consumers land first
            consts_sb = singles.tile([128, 112], mybir.dt.uint8)
            nc.sync.dma_start(out=consts_sb[:], in_=consts[:, :])
            dh_sb = consts_sb[:, 0:64].bitcast(
                FP8 if USE_FP8 else BF16).rearrange("p (t b) -> p t b", b=BSH)
            attnb_sb = consts_sb[:, 64:96].bitcast(F32)
            v_sb = consts_sb[:, 96:112].bitcast(BF16)
            ones_sb = singles.tile([1, 128], BF16)
            nc.vector.memset(ones_sb[:], 1.0)

            wd_sb = singles.tile([128, KH, A], FP8 if USE_FP8 else BF16)
            wd_dram = w_dec.rearrange("(t p) a -> p t a", p=128)
            nc.sync.dma_start(out=wd_sb[:], in_=wd_dram[:, :, :])

            we_sb = singles.tile([128, KE, A], FP8 if USE_FP8 else BF16)
            we_dram = w_enc.rearrange("(t p) a -> p t a", p=128)

            ctx_sb = singles.tile([128, BSH * KE], F32)
            dpT_sb = singles.tile([128, AT, BSH], F32)

            # --- dec_projT[a, b] = w_dec.T @ dh  (+ attn_b) --------------
            KPH = KH // 2 if USE_FP8 else KH
            for t in range(AT):
                ps = ps_misc.tile([128, BSH], F32, bufs=2)
                if USE_FP8:
                    for k in range(KPH):
                        nc.tensor.matmul(ps[:],
                                         wd_sb[:, 2 * k:2 * k + 2, ts(t, 128)],
                                         dh_sb[:, 2 * k:2 * k + 2, :],
                                         start=(k == 0), stop=(k == KPH - 1),
                                         perf_mode=DR)
                else:
                    for k in range(KH):
                        nc.tensor.matmul(ps[:], wd_sb[:, k, ts(t, 128)],
                                         dh_sb[:, k, :],
                                         start=(k == 0), stop=(k == KH - 1))
                nc.scalar.activation(out=dpT_sb[:, t, :], in_=ps[:],
                                     func=Ident, bias=attnb_sb[:, t:t + 1],
                                     scale=DESCALE if USE_FP8 else 1.0)

            # --- per-batch pipeline -------------------------------------
            # The post-softmax work of batch b (weights broadcast + context
            # reduction) is emitted in the middle of batch b+1's matmul
            # stream, so the TensorE ones-broadcast never stalls on the
            # softmax and the Vector/Scalar engines drain steadily.
            def make_context_stage(b, eb_sb, rzvec, p_bf):
                def emit():
                    # broadcast normalized attw across partitions in one PE
                    # matmul: wbc[p, s] = rzvec[p] * p_bf[s] = attw[s]
                    wbc = wbcp.tile([128, S], BF16)
                    ps_w = ps_misc.tile([128, S], F32, bufs=1)
                    nc.tensor.matmul(ps_w[:], rzvec[:1, :], p_bf[:1, :],
                                     start=True, stop=True)
                    nc.vector.tensor_copy(out=wbc[:], in_=ps_w[:])
                    # context[e] = sum_s attw[s] * enc[e, s]: most tiles use
                    # the fused multiply+reduce (scalar_tensor_tensor, 1x
                    # rate) on VectorE; the rest use a 2x-rate bf16 multiply
                    # on VectorE plus the free-dim reduction (Identity +
                    # accum_out) on ScalarE.  The last batch is the serial
                    # tail, so split it evenly.
                    n_stt = {BSH - 2: KE}.get(b, 12)
                    # emit the ScalarE-reduced tiles first so that lane
                    # starts as early as possible (matters for the tail)
                    for k in list(range(n_stt, KE)) + list(range(n_stt)):
                        acc = ctx_sb[:, b * KE + k: b * KE + k + 1]
                        scratch = sttp.tile([128, S], BF16)
                        if k < n_stt:
                            nc.vector.scalar_tensor_tensor(
                                out=scratch[:], in0=eb_sb[:, k, :], scalar=1.0,
                                in1=wbc[:], op0=mult, op1=mult, accum_out=acc)
                        else:
                            nc.vector.tensor_mul(scratch[:], eb_sb[:, k, :],
                                                 wbc[:])
                            nc.scalar.activation(out=scratch[:],
                                                 in_=scratch[:], func=Ident,
                                                 bias=0.0, scale=1.0,
                                                 accum_out=acc)
                    nc.sync.dma_start(out=out_ctx[:, ts(b, KE)],
                                      in_=ctx_sb[:, ts(b, KE)])
                return emit

            encn_sb = singles.tile([128, S // 128, E], BF16)

            def make_tail_context_stage(rz, p_bf):
                # Last batch: the TensorE is idle once its att matmuls are
                # done, and it is much faster than the Vector/Scalar lanes
                # at this reduction.  Transpose the exp row into columns
                # with four K=1 matmuls (rhs = scalar one), then contract
                # over s against a natural-layout [s, e] copy of this
                # batch's encoder; 1/Z folds into the psum-drain copies.
                def emit():
                    tp_ps = ps_misc.tile([128, 4], F32, bufs=1, tag="ps_w")
                    for st in range(4):
                        nc.tensor.matmul(tp_ps[:, st:st + 1],
                                         p_bf[0:1, ts(st, 128)],
                                         ones_sb[0:1, 0:1],
                                         start=(st == 0), stop=(st == 3),
                                         skip_group_check=True)
                    pn_col = smallp.tile([128, 4], BF16)
                    nc.vector.tensor_copy(out=pn_col[:], in_=tp_ps[:])
                    ctx7_sb = smallp.tile([1, E], F32)
                    Copy = mybir.ActivationFunctionType.Copy
                    for ce in range(4):
                        ps = ps_e.tile([128, S], F32)
                        for st in range(4):
                            nc.tensor.matmul(ps[0:1, :], pn_col[:, st:st + 1],
                                             encn_sb[:, st, ts(ce, S)],
                                             start=(st == 0), stop=(st == 3),
                                             skip_group_check=True)
                        nc.scalar.activation(out=ctx7_sb[0:1, ts(ce, S)],
                                             in_=ps[0:1, :], func=Copy,
                                             scale=rz[:])
                    nc.sync.dma_start(out=out_ctx7[:, :], in_=ctx7_sb[:])
                return emit

            pending_context = None
            for b in range(BSH):
                e8_sb = enc8p.tile([128, KE, S], FP8 if USE_FP8 else BF16)
                e8_dram = enc8[b].rearrange("(t p) s -> p t s", p=128)
                if b == 0:
                    # first batch: interleave the w_enc pair-tiles with the
                    # encoder quarters on the queue so the first a-tile's
                    # k-loop can track arrivals instead of waiting for the
                    # full weight block
                    for q in range(4):
                        nc.sync.dma_start(out=e8_sb[:, ts(q, KE // 4), :],
                                          in_=e8_dram[:, ts(q, KE // 4), :])
                        nc.sync.dma_start(out=we_sb[:, ts(q, 4), :],
                                          in_=we_dram[:, ts(q, 4), :])
                else:
                    for q in range(4):
                        nc.sync.dma_start(out=e8_sb[:, ts(q, KE // 4), :],
                                          in_=e8_dram[:, ts(q, KE // 4), :])
                if b < BSH - 1:
                    eb_sb = encbp.tile([128, KE, S], BF16)
                    eb_dram = encb[b].rearrange("(t p) s -> p t s", p=128)
                    for q in range(4):
                        nc.sync.dma_start(out=eb_sb[:, ts(q, KE // 4), :],
                                          in_=eb_dram[:, ts(q, KE // 4), :])
                else:
                    eb_sb = None
                if 3 <= b <= 6:
                    q = b - 3
                    encn_r = encn.rearrange("(st p) e -> p st e", p=128)
                    nc.sync.dma_start(out=encn_sb[:, q, :],
                                      in_=encn_r[:, q, :])

                energy_sb = energyp.tile([128, AT, S],
                                         FP8 if ATT_FP8 else BF16)
                ps_a = ps_att.tile([1, S], F32)
                for t in range(AT):
                    ps = ps_e.tile([128, S], F32)
                    if USE_FP8:
                        for k in range(KP):
                            nc.tensor.matmul(
                                ps[:], we_sb[:, 2 * k:2 * k + 2, ts(t, 128)],
                                e8_sb[:, 2 * k:2 * k + 2, :],
                                start=(k == 0), stop=(k == KP - 1),
                                perf_mode=DR)
                    else:
                        for k in range(KE):
                            nc.tensor.matmul(ps[:], we_sb[:, k, ts(t, 128)],
                                             e8_sb[:, k, :],
                                             start=(k == 0), stop=(k == KE - 1))
                    # energy = tanh(enc_proj * descale + dec_proj)
                    nc.scalar.activation(out=energy_sb[:, t, :], in_=ps[:],
                                         func=Tanh,
                                         bias=dpT_sb[:, t, b:b + 1],
                                         scale=DESCALE if USE_FP8 else 1.0)
                    # att[s] += v[a-tile] . energy[a-tile, s]
                    if ATT_FP8:
                        if t % 2 == 1:
                            tp = t // 2
                            nc.tensor.matmul(
                                ps_a[:], v_sb[:, 2 * tp:2 * tp + 2, 0:1],
                                energy_sb[:, 2 * tp:2 * tp + 2, :],
                                start=(tp == 0), stop=(tp == AT // 2 - 1),
                                perf_mode=DR, skip_group_check=True)
                    else:
                        nc.tensor.matmul(ps_a[:], v_sb[:, t:t + 1],
                                         energy_sb[:, t, :],
                                         start=(t == 0), stop=(t == AT - 1),
                                         skip_group_check=True)
                    if t == 1 and pending_context is not None:
                        pending_context()
                        pending_context = None

                # --- softmax over S on [1, 512] -------------------------
                # No max-subtraction: |att| <= sum|v_w| ~ 18, exp cannot
                # overflow in fp32, so exp reads the PSUM scores directly
                # and emits bf16 (its fp32 row-sum comes via accum_out).
                p_bf = smallp.tile([1, S], BF16)
                zs = smallp.tile([1, 1], F32)
                nc.scalar.activation(out=p_bf[:], in_=ps_a[:], func=Exp,
                                     bias=0.0,
                                     scale=(1.0 / V_SCALE) if ATT_FP8 else 1.0,
                                     accum_out=zs[:])
                rz = smallp.tile([1, 1], F32)
                nc.vector.reciprocal(out=rz[:], in_=zs[:])
                # 1/Z folded into the broadcast matmul's stationary vector
                rzvec = smallp.tile([1, 128], BF16)
                nc.vector.tensor_scalar_mul(out=rzvec[:], in0=ones_sb[:],
                                            scalar1=rz[:])
                # fp32 normalized weights only feed the DMA'd output
                pn = smallp.tile([1, S], F32)
                nc.vector.tensor_scalar_mul(out=pn[:], in0=p_bf[:], scalar1=rz[:])
                nc.sync.dma_start(out=out_attw[b:b + 1, :], in_=pn[:])

                if b < BSH - 1:
                    pending_context = make_context_stage(b, eb_sb, rzvec, p_bf)
                else:
                    pending_context = make_tail_context_stage(rz, p_bf)

            pending_context()

    nc.compile()
    return nc


_NC_CACHE = None


def _get_nc():
    global _NC_CACHE
    if _NC_CACHE is None:
        _NC_CACHE = build_nc()
    return _NC_CACHE


def _pack_dh(dh_i):
    # [BSH, H] -> [128, KH*BSH] with [p, t*BSH+b] = dh_i[b, t*128+p]
    arr = dh_i.T.reshape(KH, 128, BSH).transpose(1, 0, 2).reshape(128, KH * BSH)
    if USE_FP8:
        return np.clip(arr * ENC_SCALE, -224, 224).astype(NP_FP8, order="C")
    return arr.astype(NP_BF16, order="C")


def _stage_inputs(decoder_hidden, encoder_outputs, attn_w, attn_b, v_w):
    """Shard + lay out host-side. Returns in_maps for the 8 cores."""
    dh = np.asarray(decoder_hidden, np.float32)
    enc = np.asarray(encoder_outputs, np.float32)
    w = np.asarray(attn_w, np.float32)
    if USE_FP8:
        wd = np.clip(w[:H] * W_SCALE, -224, 224).astype(NP_FP8)    # [H, A]
    else:
        wd = w[:H].astype(NP_BF16)
    if USE_FP8:
        we = np.clip(w[H:] * W_SCALE, -224, 224).astype(NP_FP8)
    else:
        we = w[H:].astype(NP_BF16)
    ab = np.ascontiguousarray(
        np.asarray(attn_b, np.float32).reshape(AT, 128).T)      # [128, AT]
    vv = np.ascontiguousarray(
        np.asarray(v_w, np.float32).reshape(AT, 128).T).astype(NP_BF16)

    in_maps = []
    for i in range(N_CORES):
        sl = slice(i * BSH, (i + 1) * BSH)
        enc_t = enc[sl].transpose(0, 2, 1)
        if USE_FP8:
            enc8_i = np.clip(enc_t * ENC_SCALE, -224, 224).astype(
                NP_FP8, order="C")
        else:
            enc8_i = enc_t.astype(NP_BF16, order="C")
        consts_i = np.zeros((128, 112), np.uint8)
        consts_i[:, 0:64] = _pack_dh(dh[sl]).view(np.uint8)
        consts_i[:, 64:96] = ab.view(np.uint8)
        consts_i[:, 96:112] = vv.view(np.uint8)
        in_maps.append({
            "encn": enc[sl][BSH - 1].astype(NP_BF16, order="C"),
            "enc8": enc8_i,
            "encb": enc_t.astype(NP_BF16, order="C"),
            "w_dec": wd,
            "w_enc": we,
            "consts": consts_i,
        })
    return in_maps


def run(inputs, trace=False, **run_kwargs):
    """Compile (cached), execute on 8 cores, reassemble full outputs.

    Returns ((context, attention_weights), BassKernelResults).
    """
    nc = _get_nc()
    in_maps = _stage_inputs(
        inputs["decoder_hidden"], inputs["encoder_outputs"],
        inputs["attn_w"], inputs["attn_b"], inputs["v_w"])
    res = run_bass_kernel_spmd(nc, in_maps, core_ids=list(range(N_CORES)),
                               trace=trace, **run_kwargs)
    context = np.empty((B, E), np.float32)
    attw = np.empty((B, S), np.float32)
    for i, r in enumerate(res.results):
        sl = slice(i * BSH, (i + 1) * BSH)
        # out_ctx[p, b*KE + k] == context[b, k*128 + p]
        context[sl] = (r["out_ctx"].reshape(128, BSH, KE)
                       .transpose(1, 2, 0).reshape(BSH, E))
        context[i * BSH + BSH - 1] = r["out_ctx7"][0]
        attw[sl] = r["out_attw"]
    return (context, attw), res


def kernel(**inputs):
    (context, attw), _ = run(inputs, trace=False)
    return context, attw
